# revision 1
# baseline (speedup 1.0000x reference)
"""BertScore model kernel for Trainium2 (8 NeuronCores, SPMD data-parallel over B).

Reference computation (see problem): cosine-normalized per-layer token reps,
per-(layer,batch) similarity matrix dots = h1 @ h2^T (256x256, contraction
D=1024), ragged masked max over rows/cols + masked means -> s1,s2, F1
harmonic mean -> (B,NL) features, BatchNorm over batch, linear head -> (B,).

Split of work:
- Host: normalization folded into the inputs (h = r/||r||), layout transpose
  to (NL,B,D,L) so the contraction dim D lands on SBUF partitions, additive
  ragged mask rows, and the tiny (B,4) BatchNorm + head epilogue (the
  cross-device batch-stats reduction happens here at gather time).
- Device (per core, 8 batches): 32x [DMA 2 blocks -> 16 accumulating
  matmuls + K=1 mask-row matmul (adds m2[j] to every row) -> DVE max-reduce
  for the row direction -> PE transpose of the 256x256 sim matrix + K=1
  mask-row matmul (adds m1[i]) -> DVE max-reduce for the column direction],
  accumulating 128-wide max vectors into two (128,64) buffers, DMA'd out once.

Masks are applied additively (0 valid / -1e30 invalid). The m2 row added to
the sim matrix also leaks into the transposed path, but it only offsets
whole columns j: valid j columns get +0 (exact) and invalid j columns are
dropped in the host epilogue.

The matmul dtype is selectable: float16 (default; halves DMA traffic, which
is the roofline — end-to-end rel err 6.4e-5) or float32r (full fp32 storage,
fast PE mode, rel err 2.8e-5, ~2x the DMA time).
Input DMA uses a d=8p+q partition mapping so every partition reads
4KB-contiguous runs (measured 1.6x faster than the 512B-run t*128+p mapping).
Measured device time: ~295 us/iteration under a serializing device-side
For_i loop (upper bound; the For_i back-edge defeats cross-iteration
pipelining); cost-model estimate 107.9 us against a ~99 us pure-DMA floor.
"""
import os
import numpy as np

NL, B, L1, L2, D = 4, 64, 256, 256, 1024
NCORES = 8
BB = B // NCORES          # batches per core
KT = D // 128             # contraction tiles
NEG = -1.0e30             # additive mask for invalid positions
BN_EPS = 1e-8
LOGIT_SCALE = 1.0

DTYPE = os.environ.get("BSM_DTYPE", "f16")       # f16 | f32r | f32
REPEAT = int(os.environ.get("BSM_REPEAT", "1"))  # body repeats (for timing)
U = int(os.environ.get("BSM_U", "2"))            # batches merged per DMA
SKIP = set(os.environ.get("BSM_SKIP", "").split(","))  # debug: mm,act,red,dt
IOBUFS = int(os.environ.get("BSM_IOBUFS", "4"))
LOOPN = int(os.environ.get("BSM_LOOPN", "0"))  # >0: wrap body in device For_i loop

_CACHE = {}


def _build(dtype_name, repeat, u, iobufs):
    import concourse.bacc as bacc
    import concourse.bass as bass
    import concourse.mybir as mybir
    import concourse.tile as tile
    from concourse.masks import make_identity

    f32 = mybir.dt.float32
    f32r = mybir.dt.float32r
    dt_in = {
        "f32r": f32r,
        "f16": mybir.dt.float16,
        "f32": f32,
    }[dtype_name]

    nc = bacc.Bacc("TRN2", target_bir_lowering=False, debug=False,
                   num_devices=NCORES)

    h1t = nc.dram_tensor("h1t", [NL, BB, D, L1], dt_in, kind="ExternalInput")
    h2t = nc.dram_tensor("h2t", [NL, BB, D, L2], dt_in, kind="ExternalInput")
    # m1 as per-partition columns (p, b, half): m1c[p,b,h] = m1[b, h*128+p]
    m1c = nc.dram_tensor("m1c", [128, BB, 2], f32, kind="ExternalInput")
    m2d = nc.dram_tensor("m2", [BB, L2], f32r, kind="ExternalInput")
    onesd = nc.dram_tensor("ones", [1, 128], f32r, kind="ExternalInput")
    NCOL = NL * BB * 2
    rmd = nc.dram_tensor("rm", [128, NCOL], f32, kind="ExternalOutput")
    cmd = nc.dram_tensor("cm", [128, NCOL], f32, kind="ExternalOutput")

    with tile.TileContext(nc) as tc:
        with tc.tile_pool(name="consts", bufs=1) as consts, \
             tc.tile_pool(name="io", bufs=iobufs) as io, \
             tc.tile_pool(name="dsbp", bufs=4) as dsbp, \
             tc.tile_pool(name="accp", bufs=1) as accp, \
             tc.tile_pool(name="ps", bufs=3, space="PSUM") as ps, \
             tc.tile_pool(name="psT", bufs=2, space="PSUM") as psT:

            ident = consts.tile([128, 128], f32)
            make_identity(nc, ident)
            ones = consts.tile([1, 128], f32r)
            nc.sync.dma_start(out=ones, in_=onesd.ap())

            # m2 mask rows, one partition: (1, BB, L2); m1 as columns (128, BB, 2)
            m2sb = consts.tile([1, BB, L2], f32r)
            m2ap = m2d.ap()
            nc.sync.dma_start(out=m2sb, in_=bass.AP(
                tensor=m2ap.tensor, offset=m2ap.offset,
                ap=[[0, 1], [L2, BB], [1, L2]]))
            m1sb = consts.tile([128, BB, 2], f32)
            nc.sync.dma_start(out=m1sb, in_=m1c.ap())

            RM = accp.tile([128, NCOL], f32)
            CM = accp.tile([128, NCOL], f32)
            if SKIP & {"mm", "act", "red", "dt"}:
                nc.vector.memset(RM, 0.0)
                nc.vector.memset(CM, 0.0)

            h1ap = h1t.ap()
            h2ap = h2t.ap()
            vmax = mybir.AluOpType.max
            X = mybir.AxisListType.X
            IDENT = mybir.ActivationFunctionType.Identity

            import contextlib
            loop_cm = (tc.For_i(0, LOOPN, 1,
                                hint_engines=(mybir.EngineType.PE,))
                       if LOOPN > 0 else contextlib.nullcontext())
            with loop_cm:
              for _rep in range(repeat):
                for l in range(NL):
                    # d = 8p + q: partition p reads 4KB-contiguous (q, i)
                    src1 = h1ap[l].rearrange("b (p q) i -> p b (q i)", p=128)
                    src2 = h2ap[l].rearrange("b (p q) j -> p b (q j)", p=128)
                    for bu in range(BB // u):
                        h1blk = io.tile([128, u, KT * L1], dt_in, tag="h1")
                        nc.sync.dma_start(
                            out=h1blk, in_=src1[:, bu * u:(bu + 1) * u, :])
                        h2blk = io.tile([128, u, KT * L2], dt_in, tag="h2")
                        nc.sync.dma_start(
                            out=h2blk, in_=src2[:, bu * u:(bu + 1) * u, :])
                        h1v = h1blk.rearrange("p u (q i) -> p u q i", q=KT)
                        h2v = h2blk.rearrange("p u (q j) -> p u q j", q=KT)

                        for ul in range(u):
                            if "mm" in SKIP:
                                continue
                            b = bu * u + ul
                            dsbs = []
                            for it in range(2):
                                dps = ps.tile([128, L2], f32, tag=f"dots{it}")
                                for k in range(KT):
                                    nc.tensor.matmul(
                                        out=dps,
                                        lhsT=h1v[:, ul, k,
                                                  it * 128:(it + 1) * 128],
                                        rhs=h2v[:, ul, k, :],
                                        start=(k == 0), stop=False)
                                # += m2[j] on every row (K=1 accumulate)
                                nc.tensor.matmul(out=dps, lhsT=ones,
                                                 rhs=m2sb[:, b, :],
                                                 start=False, stop=True)
                                # copy PSUM->SBUF with per-partition m1[i]
                                # added (ACT): dsb = dps + m1[i]
                                if "act" in SKIP:
                                    continue
                                dsb = dsbp.tile([128, L2], f32, tag=f"dsb{it}")
                                nc.scalar.activation(
                                    out=dsb, in_=dps, func=IDENT,
                                    bias=m1sb[:, b, it:it + 1])
                                dsbs.append(dsb)
                                # row max: m1[i] is constant along j, so the
                                # masked copy gives the same max for valid i
                                if "red" not in SKIP:
                                    col = (l * BB + b) * 2 + it
                                    nc.vector.tensor_reduce(
                                        out=RM[:, col:col + 1], in_=dsb,
                                        axis=X, op=vmax)

                            if "dt" in SKIP:
                                continue
                            dT = psT.tile([128, 2, L1], f32, tag="dT")
                            for jt in range(2):
                                for it in range(2):
                                    nc.tensor.transpose(
                                        out=dT[:, jt, it * 128:(it + 1) * 128],
                                        in_=dsbs[it][:, jt * 128:(jt + 1) * 128],
                                        identity=ident)
                            for jt in range(2):
                                col = (l * BB + b) * 2 + jt
                                nc.vector.tensor_reduce(
                                    out=CM[:, col:col + 1], in_=dT[:, jt, :],
                                    axis=X, op=vmax)

            for l in range(NL):
                c0, c1 = l * BB * 2, (l + 1) * BB * 2
                nc.sync.dma_start(out=rmd.ap()[:, c0:c1], in_=RM[:, c0:c1])
                nc.sync.dma_start(out=cmd.ap()[:, c0:c1], in_=CM[:, c0:c1])

    nc.finalize()
    return nc


def _get_nc():
    key = (DTYPE, REPEAT, U, IOBUFS, LOOPN, tuple(sorted(SKIP)))
    if key not in _CACHE:
        _CACHE[key] = _build(*key[:4])
    return _CACHE[key]


def _host_prep(reps1, reps2, len1, len2):
    """Normalize, transpose to (NL,B,D,L), build masks; returns per-core maps."""
    np_in = np.float16 if DTYPE == "f16" else np.float32

    def prep(r):
        r = np.asarray(r, dtype=np.float32)
        n = np.sqrt(np.einsum('lbid,lbid->lbi', r, r))
        h = r / n[..., None]
        return np.ascontiguousarray(h.transpose(0, 1, 3, 2)).astype(np_in)

    h1t = prep(reps1)   # (NL, B, D, L1)
    h2t = prep(reps2)
    len1 = np.asarray(len1).astype(np.int64)
    len2 = np.asarray(len2).astype(np.int64)
    ar1 = np.arange(L1)[None, :]
    ar2 = np.arange(L2)[None, :]
    m1 = np.where(ar1 < len1[:, None], 0.0, NEG).astype(np.float32)  # (B, L1)
    m2 = np.where(ar2 < len2[:, None], 0.0, NEG).astype(np.float32)
    # (B, L1) -> (B, 2, 128) -> (128, B, 2)
    m1c = np.ascontiguousarray(m1.reshape(B, 2, 128).transpose(2, 0, 1))

    in_maps = []
    for c in range(NCORES):
        sl = slice(c * BB, (c + 1) * BB)
        in_maps.append({
            "h1t": np.ascontiguousarray(h1t[:, sl]),
            "h2t": np.ascontiguousarray(h2t[:, sl]),
            "m1c": np.ascontiguousarray(m1c[:, sl]),
            "m2": np.ascontiguousarray(m2[sl]),
            "ones": np.ones((1, 128), dtype=np.float32),
        })
    return in_maps, len1, len2


def _epilogue(results, len1, len2, w, b):
    """rm/cm (128, NL*BB*2) per core -> s1,s2 -> F1 -> BatchNorm -> head."""
    maxv_rows = np.empty((NL, B, L1), dtype=np.float64)  # max over valid j, per i
    maxv_cols = np.empty((NL, B, L2), dtype=np.float64)  # max over valid i, per j
    for c, res in enumerate(results):
        rm = np.asarray(res["rm"], dtype=np.float64)  # (128, NCOL)
        cm = np.asarray(res["cm"], dtype=np.float64)
        # column t = (l*BB + b)*2 + half ; partition p -> index half*128 + p
        rm_r = rm.T.reshape(NL, BB, 2, 128).reshape(NL, BB, 256)
        cm_r = cm.T.reshape(NL, BB, 2, 128).reshape(NL, BB, 256)
        maxv_rows[:, c * BB:(c + 1) * BB] = rm_r
        maxv_cols[:, c * BB:(c + 1) * BB] = cm_r

    ar1 = np.arange(L1)[None, :]
    ar2 = np.arange(L2)[None, :]
    mask1 = (ar1 < len1[:, None])  # (B, L1)
    mask2 = (ar2 < len2[:, None])
    n1 = len1.astype(np.float64)
    n2 = len2.astype(np.float64)

    # s2: mean over valid i of (max over valid j); s1: mean over valid j of
    # (max over valid i)
    s2 = np.where(mask1[None], maxv_rows, 0.0).sum(axis=2) / n1[None]  # (NL, B)
    s1 = np.where(mask2[None], maxv_cols, 0.0).sum(axis=2) / n2[None]
    feat = (2.0 * s1 * s2 / (s1 + s2)).T                    # (B, NL)
    mean = feat.mean(axis=0, keepdims=True)
    var = ((feat - mean) ** 2).mean(axis=0, keepdims=True)
    feat = (feat - mean) / np.sqrt(var + BN_EPS)
    w = np.asarray(w, dtype=np.float64)
    bb = np.asarray(b, dtype=np.float64)
    out = LOGIT_SCALE * (feat @ w.T + bb)[:, 0]
    return out.astype(np.float32)


LAST_RUN = {}


def kernel(reps1, reps2, len1, len2, w, b):
    from concourse.bass_utils import run_bass_kernel_spmd

    nc = _get_nc()
    in_maps, l1, l2 = _host_prep(reps1, reps2, len1, len2)
    res = run_bass_kernel_spmd(nc, in_maps, list(range(NCORES)))
    LAST_RUN["results"] = res
    LAST_RUN["in_maps"] = in_maps
    return _epilogue(res.results, l1, l2, w, b)



# revision 46
# speedup vs baseline: 5.5893x; 5.5893x over previous
"""BertScore model kernel for Trainium2 (8 NeuronCores, SPMD data-parallel over B).

Reference: cosine-normalized per-layer token reps, per-(layer,batch) similarity
matrix dots = h1 @ h2^T (ragged L1 x L2, contraction D=1024), masked max over
rows/cols + masked means -> s1,s2, F1 harmonic mean -> (B,NL) features,
BatchNorm over batch, linear head -> (B,).

Ragged specialization: sequence lengths are inputs, known before the device
program is built, and average ~L/2 -- so the kernel is COMPILED FOR THE
LENGTHS. SPMD needs one program for all 8 cores, so the 64 batches are
partitioned into 8 slots x 8 cores by a seeded local search (with optional
per-batch orientation flip, h1<->h2) minimizing the slot-padded byte count;
every core's slot k is padded to the same (A[k], C[k]) token counts with
duplicates of token 0 (duplicates never change a max; the host epilogue sums
only truly-valid entries). This cuts DMA bytes ~1.55x and PE/DVE work ~2x
vs computing the full 256x256 matrices.

Device kernel per (layer, slot): i-side tokens are PE-stationary exactly once
  - ceil(A/128) x 4 fp8e4m3 DoubleRow matmuls (K=256 each) -> psD in PSUM
  - ACT copy psD (f32) -> dsb (f16 SBUF)
  - DVE row-max reduce straight from psD (PSUM f32) into RM
  - f16 PE transposes dsb -> psT (f16 PSUM), deferred one slot so the
    in-order PE queue never waits on the ACT copy
  - DVE col-max reduce psT -> CM
(gpsimd/Pool cannot run TensorTensor ops or read PSUM on TRN2; all max work
is DVE at ~1-1.5 cyc/elem.) BatchNorm cancels any uniform scale, so h is
scaled by 16 into fp8e4m3's normal range (end-to-end rel err ~8e-3 vs the
2e-2 gate). Host: normalize/scale/quantize, slot packing, partition-major
transpose (d = 8p + q), and the tiny BatchNorm+head epilogue.
"""
import os
import numpy as np

NL, B, L1MAX, D = 4, 64, 256, 1024
NCORES = 8
BB = B // NCORES          # batches per core = number of slots
QD = 8                    # d = 8p + q rows per partition
SCALE = 16.0              # fp8 range scaling; cancelled by BatchNorm
BN_EPS = 1e-8
LOGIT_SCALE = 1.0

IOBUFS = int(os.environ.get("BSM_IOBUFS", "3"))
PSBUFS = int(os.environ.get("BSM_PSBUFS", "3"))
DSBUFS = int(os.environ.get("BSM_DSBUFS", "4"))

_CACHE = {}


def _plan(len1, len2):
    """Partition 64 batches into BB slots x NCORES cores (with orientation
    flips) minimizing slot-padded DMA bytes. Deterministic (seeded)."""
    import random
    rng = random.Random(12345)
    L1a = [int(x) for x in len1]
    L2a = [int(x) for x in len2]

    def ilen(b, f):
        return L2a[b] if f else L1a[b]

    def jlen(b, f):
        return L1a[b] if f else L2a[b]

    def slot_terms(slot):
        """(dma, pe, dve) device-ns models for one slot (per layer)."""
        a = -(-max(ilen(b, f) for b, f in slot) // 32) * 32
        c = -(-max(jlen(b, f) for b, f in slot) // 32) * 32
        nb1 = -(-a // 128)
        nb2 = -(-c // 128)
        dma = (a + c) * 2.93
        pe = (nb1 * 4 * (c / 2 + 128) + nb1 * nb2 * 256) * 0.4167
        dve = (nb1 * c + nb2 * a) * 1.042 + 250
        return dma, pe, dve

    def objective(slots):
        d = p = v = 0.0
        for s in slots:
            dd, pp, vv = slot_terms(s)
            d += dd
            p += pp
            v += vv
        return max(d, p, v) + 0.05 * (d + p + v)

    items = [(b, L1a[b] < L2a[b]) for b in range(B)]
    items.sort(key=lambda bf: -max(L1a[bf[0]], L2a[bf[0]]))
    slots = [items[NCORES * k:NCORES * (k + 1)] for k in range(BB)]
    cur = objective(slots)
    for _ in range(80000):
        k1 = rng.randrange(BB)
        k2 = rng.randrange(BB)
        if k1 == k2:
            i = rng.randrange(NCORES)
            b, f = slots[k1][i]
            slots[k1][i] = (b, not f)
            new = objective(slots)
            if new <= cur:
                cur = new
            else:
                slots[k1][i] = (b, f)
        else:
            i1 = rng.randrange(NCORES)
            i2 = rng.randrange(NCORES)
            slots[k1][i1], slots[k2][i2] = slots[k2][i2], slots[k1][i1]
            new = objective(slots)
            if new <= cur:
                cur = new
            else:
                slots[k1][i1], slots[k2][i2] = slots[k2][i2], slots[k1][i1]

    # biggest slots early (small ones taper the tail), but lead with the
    # smallest so the first matmul starts on minimal data; A and C rounded
    # to multiples of 32 (PE ldweights ISA granularity + PSUM alignment)
    slots.sort(key=lambda s: -sum(slot_terms(s)))
    slots = slots[-1:] + slots[:-1]
    A = [-(-max(ilen(b, f) for b, f in s) // 16) * 16 for s in slots]
    C = [-(-max(jlen(b, f) for b, f in s) // 16) * 16 for s in slots]
    # core assignment within a slot is arbitrary: member m -> core m
    return slots, A, C


def _layout(A, C):
    """Static program layout derived from padded slot lengths."""
    nb1 = [-(-a // 128) for a in A]
    nb2 = [-(-c // 128) for c in C]
    offA = np.concatenate([[0], np.cumsum(A)]).astype(int)      # i tokens
    offC = np.concatenate([[0], np.cumsum(C)]).astype(int)      # j tokens
    # ragged RM/CM column offsets per (l, slot)
    rmoff = np.zeros((NL, BB + 1), dtype=int)
    cmoff = np.zeros((NL, BB + 1), dtype=int)
    r = c = 0
    for l in range(NL):
        for k in range(BB):
            rmoff[l, k] = r
            cmoff[l, k] = c
            r += nb1[k]
            c += nb2[k]
        rmoff[l, BB] = r
        cmoff[l, BB] = c
    # DMA chunk groups of slots: small lead, then ~6KB/partition groups.
    # h1+h2 are fused into ONE dram stream laid out group-contiguously
    # (h1 slots then h2 slots within each group) so each (layer, group)
    # is a single DMA instruction -- SP issue time is the scarce resource.
    groups, g = [], []
    gbytes = 0
    for k in range(BB):
        g.append(k)
        gbytes += (A[k] + C[k]) * QD
        if (len(groups) < 2 and len(g) >= 1) or gbytes >= 6144 \
                or k == BB - 1:
            groups.append(g)
            g, gbytes = [], 0
    if g:
        groups.append(g)
    # token offsets of each slot's h1/h2 segment in the fused stream
    hoff1 = [0] * BB
    hoff2 = [0] * BB
    goff = [0] * (len(groups) + 1)
    t = 0
    for gi, g in enumerate(groups):
        goff[gi] = t
        for k in g:
            hoff1[k] = t
            t += A[k]
        for k in g:
            hoff2[k] = t
            t += C[k]
    goff[len(groups)] = t
    return nb1, nb2, offA, offC, rmoff, cmoff, groups, hoff1, hoff2, goff


def _build(A, C, iobufs, psbufs, dsbufs):
    import concourse.bacc as bacc
    import concourse.bass as bass
    import concourse.mybir as mybir
    import concourse.tile as tile
    from concourse.masks import make_identity

    f32 = mybir.dt.float32
    f16 = mybir.dt.float16
    fp8 = mybir.dt.float8e4
    DR = mybir.MatmulPerfMode.DoubleRow

    (nb1, nb2, offA, offC, rmoff, cmoff, groups,
     hoff1, hoff2, goff) = _layout(A, C)
    ST = int(goff[-1])   # total fused tokens (h1 + h2)
    RMN, CMN = int(rmoff[-1, -1]), int(cmoff[-1, -1])

    nc = bacc.Bacc("TRN2", target_bir_lowering=False, debug=False,
                   num_devices=NCORES, enable_partition_id=False)

    # [l, p, packed tokens * 8q] -- per (l, p) group/slot-contiguous runs
    hhd = nc.dram_tensor("hh", [NL, 128, ST * QD], fp8, kind="ExternalInput")
    rmd = nc.dram_tensor("rm", [128, RMN], f16, kind="ExternalOutput")
    cmd = nc.dram_tensor("cm", [128, CMN], f16, kind="ExternalOutput")

    vmax = mybir.AluOpType.max
    X = mybir.AxisListType.X
    IDENT = mybir.ActivationFunctionType.Identity

    with tile.TileContext(nc) as tc:
        with tc.tile_pool(name="consts", bufs=1) as consts, \
             tc.tile_pool(name="io", bufs=iobufs) as io, \
             tc.tile_pool(name="dsbp", bufs=dsbufs) as dsbp, \
             tc.tile_pool(name="accp", bufs=1) as accp, \
             tc.tile_pool(name="ps", bufs=psbufs, space="PSUM") as ps, \
             tc.tile_pool(name="psT", bufs=psbufs, space="PSUM") as psT:

            ident = consts.tile([128, 128], f16)
            make_identity(nc, ident)
            RM = accp.tile([128, RMN], f16)
            CM = accp.tile([128, CMN], f16)

            def finish_slot(st):
                """Transposes + col reduction for a completed slot (deferred
                one slot so the PE queue isn't blocked on the ACT copy)."""
                dsb, l, k = st
                a, ce = A[k], C[k]
                psTt = psT.tile([128, 512], f16, tag="psT")
                for jb in range(nb2[k]):
                    jl = min(128, ce - jb * 128)
                    for ib in range(nb1[k]):
                        il = min(128, a - ib * 128)
                        nc.tensor.transpose(
                            out=psTt[0:jl, a * jb + ib * 128:
                                     a * jb + ib * 128 + il],
                            in_=dsb[0:il, ce * ib + jb * 128:
                                    ce * ib + jb * 128 + jl],
                            identity=ident[0:il, 0:il])
                co = int(cmoff[l, k])
                nc.vector.tensor_reduce(
                    out=CM[:, co:co + nb2[k]],
                    in_=psTt[:, 0:nb2[k] * a].rearrange(
                        "p (n a) -> p n a", n=nb2[k]),
                    axis=X, op=vmax)

            pending = None
            for l in range(NL):
                for gi, g in enumerate(groups):
                    t0, t1 = int(goff[gi]), int(goff[gi + 1])
                    hblk = io.tile([128, (t1 - t0) * QD], fp8,
                                   tag=f"hg{gi}")
                    nc.sync.dma_start(
                        out=hblk, in_=hhd.ap()[l][:, t0 * QD:t1 * QD])

                    for k in g:
                        a, ce = A[k], C[k]
                        h1v = hblk[:, (hoff1[k] - t0) * QD:
                                   (hoff1[k] - t0 + a) * QD].rearrange(
                            "p (q i) -> p q i", q=QD)
                        h2v = hblk[:, (hoff2[k] - t0) * QD:
                                   (hoff2[k] - t0 + ce) * QD].rearrange(
                            "p (q j) -> p q j", q=QD)
                        # psD[ib][i, j], i = ib*128 + p (flat [128, nb1*ce])
                        psD = ps.tile([128, 512], f32, tag="psD")
                        for ib in range(nb1[k]):
                            il = min(128, a - ib * 128)
                            for t in range(QD // 2):
                                nc.tensor.matmul(
                                    out=psD[0:il, ce * ib:ce * ib + ce],
                                    lhsT=h1v[:, 2 * t:2 * t + 2,
                                             ib * 128:ib * 128 + il],
                                    rhs=h2v[:, 2 * t:2 * t + 2, :],
                                    start=(t == 0), stop=(t == 3),
                                    perf_mode=DR)
                        dsb = dsbp.tile([128, 512], f16, tag="dsb")
                        nc.scalar.activation(
                            out=dsb[:, 0:nb1[k] * ce],
                            in_=psD[:, 0:nb1[k] * ce], func=IDENT)
                        ro = int(rmoff[l, k])
                        nc.vector.tensor_reduce(
                            out=RM[:, ro:ro + nb1[k]],
                            in_=psD[:, 0:nb1[k] * ce].rearrange(
                                "p (n c) -> p n c", n=nb1[k]),
                            axis=X, op=vmax)
                        if pending is not None:
                            finish_slot(pending)
                        pending = (dsb, l, k)
            if pending is not None:
                finish_slot(pending)

            nc.sync.dma_start(out=rmd.ap(), in_=RM)
            nc.sync.dma_start(out=cmd.ap(), in_=CM)

    nc.finalize()
    return nc


def _get_nc(A, C):
    key = (tuple(A), tuple(C), IOBUFS, PSBUFS, DSBUFS)
    if key not in _CACHE:
        _CACHE[key] = _build(A, C, IOBUFS, PSBUFS, DSBUFS)
    return _CACHE[key]


def _host_prep(reps1, reps2, len1, len2, slots, A, C):
    """Normalize+scale+quantize to fp8, pack slot tokens (pad = dups of
    token 0), transpose to (NL, 128p, tokens*8q); per-core input maps."""
    import ml_dtypes
    fp8 = ml_dtypes.float8_e4m3

    def norm(r):
        r = np.asarray(r, dtype=np.float32)
        n = np.sqrt(np.einsum('lbid,lbid->lbi', r, r))
        return (r * (SCALE / n[..., None])).astype(fp8)   # (NL, B, L, D)

    h1 = norm(reps1)
    h2 = norm(reps2)
    (nb1, nb2, offA, offC, rmoff, cmoff, groups,
     hoff1, hoff2, goff) = _layout(A, C)
    ST = int(goff[-1])
    in_maps = []
    for c in range(NCORES):
        pp = np.empty((NL, ST, D), dtype=fp8)    # fused h1|h2 token stream
        for k, slot in enumerate(slots):
            b, f = slot[c]
            hi, hj = (h2, h1) if f else (h1, h2)
            iv = int(len2[b] if f else len1[b])   # valid i tokens
            jv = int(len1[b] if f else len2[b])   # valid j tokens
            o1, o2 = hoff1[k], hoff2[k]
            pp[:, o1:o1 + A[k]] = hi[:, b, 0:1]   # token-0 fill (pad)
            pp[:, o2:o2 + C[k]] = hj[:, b, 0:1]
            pp[:, o1:o1 + iv] = hi[:, b, :iv]
            pp[:, o2:o2 + jv] = hj[:, b, :jv]

        # per segment: (NL, n, D) -> (NL, D, n) -> (NL, 128p, 8q*n);
        # segments concatenated so each slot's per-partition run is
        # contiguous and q-major (matches the device "(q i)" rearrange)
        segs = []
        bounds = sorted(set(list(hoff1) + list(hoff2) + [ST]))
        for s0, s1 in zip(bounds[:-1], bounds[1:]):
            seg = pp[:, s0:s1].transpose(0, 2, 1)   # (NL, D, n)
            segs.append(seg.reshape(NL, 128, QD * (s1 - s0)))
        in_maps.append({"hh": np.ascontiguousarray(
            np.concatenate(segs, axis=2))})
    return in_maps


LAST_RUN = {}


def kernel(reps1, reps2, len1, len2, w, b):
    from concourse.bass_utils import run_bass_kernel_spmd

    len1 = np.asarray(len1).astype(np.int64)
    len2 = np.asarray(len2).astype(np.int64)
    slots, A, C = _plan(len1, len2)
    nc = _get_nc(A, C)
    in_maps = _host_prep(reps1, reps2, len1, len2, slots, A, C)
    res = run_bass_kernel_spmd(nc, in_maps, list(range(NCORES)))
    LAST_RUN["results"] = res
    LAST_RUN["in_maps"] = in_maps
    LAST_RUN["plan"] = (slots, A, C)

    nb1 = [-(-a // 128) for a in A]
    nb2 = [-(-c // 128) for c in C]
    rmoff = np.zeros((NL, BB), dtype=int)
    cmoff = np.zeros((NL, BB), dtype=int)
    r = c = 0
    for l in range(NL):
        for k in range(BB):
            rmoff[l, k] = r
            cmoff[l, k] = c
            r += nb1[k]
            c += nb2[k]

    # decode: device RM = max over j per i (partition i = ib*128+p);
    # device CM = max over i per j. For flipped members these map to the
    # reference's col/row maxes respectively.
    maxv_rows = np.zeros((NL, B, L1MAX), dtype=np.float64)
    maxv_cols = np.zeros((NL, B, L1MAX), dtype=np.float64)
    for core, res_c in enumerate(res.results):
        rm = np.asarray(res_c["rm"], dtype=np.float64)  # (128, RMN)
        cm = np.asarray(res_c["cm"], dtype=np.float64)
        for k, slot in enumerate(slots):
            bidx, f = slot[core]
            ilen = int(len2[bidx] if f else len1[bidx])
            jlen = int(len1[bidx] if f else len2[bidx])
            for l in range(NL):
                imax = np.concatenate(
                    [rm[:, rmoff[l, k] + ib] for ib in range(nb1[k])])[:ilen]
                jmax = np.concatenate(
                    [cm[:, cmoff[l, k] + jb] for jb in range(nb2[k])])[:jlen]
                if f:
                    maxv_cols[l, bidx, :ilen] = imax
                    maxv_rows[l, bidx, :jlen] = jmax
                else:
                    maxv_rows[l, bidx, :ilen] = imax
                    maxv_cols[l, bidx, :jlen] = jmax

    ar = np.arange(L1MAX)[None, :]
    mask1 = (ar < len1[:, None])
    mask2 = (ar < len2[:, None])
    n1 = len1.astype(np.float64)
    n2 = len2.astype(np.float64)
    s2 = np.where(mask1[None], maxv_rows, 0.0).sum(axis=2) / n1[None]
    s1 = np.where(mask2[None], maxv_cols, 0.0).sum(axis=2) / n2[None]
    feat = (2.0 * s1 * s2 / (s1 + s2)).T
    mean = feat.mean(axis=0, keepdims=True)
    var = ((feat - mean) ** 2).mean(axis=0, keepdims=True)
    feat = (feat - mean) / np.sqrt(var + BN_EPS)
    w = np.asarray(w, dtype=np.float64)
    bb = np.asarray(b, dtype=np.float64)
    out = LOGIT_SCALE * (feat @ w.T + bb)[:, 0]
    return out.astype(np.float32)
